# revision 32
# baseline (speedup 1.0000x reference)
"""Trainium2 Bass kernel: MeanHinAggregator (GNN message passing).

Reference computation (per batch-head element bh):
    z_r  = mean_n(x_neigh_r[bh, n, :]) @ w_neigh_r          (r = 0, 1)
    out  = relu(concat(x_self[bh] @ w_self, (z0 + z1) / 2) + b)

Strategy (pure data parallel over 8 NeuronCores, batch axis sharded):
  * Per core: B_shard=128, H=10 -> 1280 rows, processed in 10 groups of 128.
  * Neighbour tiles are DMA'd in natural layout [128 bh-part, (n f) free]
    (16 KiB contiguous per partition -> full DMA bandwidth).  Loads are
    prefetched PREFETCH groups deep so the SDMA queues never drain; this is
    a memory-bound kernel (43.9 MB/core vs ~358 GB/s HBM/core -> ~123 us
    floor) so the whole game is keeping DMA streaming at full rate.
  * The mean-over-neighbours reduction: five in-place strided adds on the
    Vector engine fold the 32 neighbour slices to 1 (fp32 matmuls pay a
    double-pass on the PE, so DVE folding is cheaper per element), then one
    transposing matmul per operand (lhsT = data, rhs = identity) lands the
    sums in PSUM in the [f, bh] layout the projection matmuls need as lhsT.
  * Engine discipline to keep every queue free of head-of-line blocking:
      - DVE runs ONLY the folds.
      - The PSUM->SBUF copies run on the Scalar (ACT) engine.
      - Phase 2 of group g (projections + bias + ReLU + store) is issued one
        iteration later, after phase 1 of group g+1, so the PE queue never
        waits on a copy that itself waits on the PE.
      - No gpsimd: SWDGE would fight DVE for the shared SBUF port pair.
  * Projection: out[bh, d] = lhsT(sumT).T @ w.  The 1/(N*NR) mean scaling is
    folded into host-prescaled copies of w_neigh_*.  Bias is added with a
    K=1 matmul (lhsT = ones row, rhs = bias row) accumulating into PSUM.
  * Final ReLU on the Scalar engine (PSUM -> SBUF), then DMA out on the
    scalar HWDGE ring (sync ring carries t0/ts, scalar ring t1/stores).
"""

import numpy as np

import concourse.bacc as bacc
import concourse.bass as bass
import concourse.tile as tile
from concourse import bass_utils, mybir
from concourse._compat import with_exitstack

B, H, N, F = 1024, 10, 32, 128
HALF = 128
D = 2 * HALF
NR = 2
NCORES = 8
BSH = B // NCORES        # 128 batch rows per core
BH = BSH * H             # 1280 (bh rows per core)
GROUP = 128              # bh rows per group
F32 = mybir.dt.float32
BF16 = mybir.dt.bfloat16
PREFETCH = 5             # groups of loads kept in flight


@with_exitstack
def _tile_kernel(ctx, tc, outs, ins, ngroups):
    nc = tc.nc
    xn0, xn1, xs, cpack, bvec = ins
    (out_d,) = outs

    const = ctx.enter_context(tc.tile_pool(name="const", bufs=1))
    xpool = ctx.enter_context(tc.tile_pool(name="xp", bufs=PREFETCH))
    tbpool = ctx.enter_context(tc.tile_pool(name="tb", bufs=3))
    spool = ctx.enter_context(tc.tile_pool(name="sp", bufs=3))
    opool = ctx.enter_context(tc.tile_pool(name="op", bufs=3))
    ppool = ctx.enter_context(tc.tile_pool(name="ps", bufs=3, space="PSUM"))
    pout = ctx.enter_context(tc.tile_pool(name="po", bufs=3, space="PSUM"))

    def issue_loads(g):
        """Queue group g's neighbour loads on the two HWDGE rings (sync +
        scalar), alternating the stream<->ring pairing per group so the ACT
        sequencer's slower boot doesn't systematically delay one stream.
        Group 0 is split into half-tile transfers: its fold can then start
        on the first half while the second is still in flight (the ~4 us
        DMA completion latency is exposed only at startup)."""
        r = slice(g * GROUP, (g + 1) * GROUP)
        ea, eb = (nc.sync, nc.scalar) if g % 2 == 0 else (nc.scalar, nc.sync)
        t0 = xpool.tile([128, N * F], F32, tag="t0")
        t1 = xpool.tile([128, N * F], F32, tag="t1")
        hf = 16 * F
        ea.dma_start(t0[:, 0:hf], xn0[r, 0:hf])
        eb.dma_start(t1[:, 0:hf], xn1[r, 0:hf])
        ea.dma_start(t0[:, hf:2 * hf], xn0[r, hf:2 * hf])
        eb.dma_start(t1[:, hf:2 * hf], xn1[r, hf:2 * hf])
        return t0, t1

    # The very first thing on each ring is a fat 2 MiB load: the kernel's
    # critical path starts with group 0's data.  Everything small (x_self,
    # weights, identity, bias) is queued behind it and still lands well
    # before its first use.  x_self is loaded ONCE for all groups: its rows
    # are only 512 B per partition, and per-group 64 KiB transfers of 512 B
    # descriptors are descriptor-dominated (measured 9 us for 64 KiB when
    # contending with fat loads) — one batched transfer pays that cost once.
    inflight = [issue_loads(0)]

    # Everything on the PE path is bf16: fp32 matmuls cost a double
    # LDWEIGHTS+MATMUL pass, bf16 is single-pass.  The fp32 consts are
    # cast to bf16 once on the Scalar engine at startup (rel tolerance is
    # 2e-2; bf16 rounding contributes ~0.5%).
    #
    # All small transfers (packed consts, bias, x_self) go on the gpsimd
    # SWDGE queue: a third DMA queue that drains concurrently with the two
    # fat HWDGE rings.  On the rings, their small descriptors cost ~10 us
    # of ring time each while contending with 16 KiB-descriptor streams
    # (measured).  DVE is idle at startup, so SWDGE descriptor emission is
    # not port-blocked.
    cp = const.tile([128, 4 * 128], F32, tag="cpack")
    nc.gpsimd.dma_start(cp[:], cpack[:])
    ident = cp[:, 0:128]
    wS_t = cp[:, 128:256]
    w0_t = cp[:, 256:384]
    w1_t = cp[:, 384:512]
    b_t = const.tile([1, D], F32, tag="b")
    nc.gpsimd.dma_start(b_t[:], bvec[:])
    xsall = const.tile([128, BH // GROUP, F], F32, tag="xsall")
    nc.gpsimd.dma_start(xsall[:], xs.rearrange("(g p) f -> p g f", p=GROUP))

    inflight += [issue_loads(g) for g in range(1, min(PREFETCH, ngroups))]

    # bf16 casts AFTER the prefetch load issues: the ACT engine is also the
    # HWDGE sequencer for one ring, and these casts wait ~20 us for the
    # SWDGE consts to land — issued earlier they head-of-line block that
    # ring's load issues (measured 12 us of ring delay).
    identb = const.tile([128, 128], BF16, tag="identb")
    nc.scalar.copy(identb[:], ident)
    wSb = const.tile([128, HALF], BF16, tag="wSb")
    nc.scalar.copy(wSb[:], wS_t)
    w0b = const.tile([128, HALF], BF16, tag="w0b")
    nc.scalar.copy(w0b[:], w0_t)
    w1b = const.tile([128, HALF], BF16, tag="w1b")
    nc.scalar.copy(w1b[:], w1_t)
    bb = const.tile([1, D], BF16, tag="bb")
    nc.scalar.copy(bb[:], b_t[:])
    xsb = const.tile([128, BH // GROUP, F], BF16, tag="xsb")
    nc.scalar.copy(xsb[:], xsall[:])
    ones_t = const.tile([1, 128], BF16, tag="ones")
    nc.vector.memset(ones_t[:], 1.0)
    state = {}

    def phase2(h):
        """Projections + bias + ReLU + store for group h (issued one
        iteration late so the PE queue never stalls on the ACT copy)."""
        sacc = state.pop(h)
        r = slice(h * GROUP, (h + 1) * GROUP)
        po = pout.tile([128, D], F32, tag="po")
        nc.tensor.matmul(po[:, 0:HALF], sacc[:, 256:384], wSb[:],
                         start=True, stop=False)
        nc.tensor.matmul(po[:, 0:HALF], ones_t[:], bb[:, 0:HALF],
                         start=False, stop=True)
        nc.tensor.matmul(po[:, HALF:D], sacc[:, 0:128], w0b[:],
                         start=True, stop=False)
        nc.tensor.matmul(po[:, HALF:D], sacc[:, 128:256], w1b[:],
                         start=False, stop=False)
        nc.tensor.matmul(po[:, HALF:D], ones_t[:], bb[:, HALF:D],
                         start=False, stop=True)
        ob = opool.tile([128, D], F32, tag="ob")
        nc.scalar.activation(ob[:], po[:], mybir.ActivationFunctionType.Relu)
        # Mid-run stores go on the SWDGE queue: 1 KiB-descriptor stores cost
        # ~2 us of HWDGE ring time each and delay every later load.  The
        # last two stores happen after the load streams have drained, where
        # the HWDGE rings are empty and lower-latency than SWDGE emission.
        if h >= BH // GROUP - 2:
            eng = nc.sync if h % 2 == 0 else nc.scalar
            eng.dma_start(out_d[r, :], ob[:])
        else:
            nc.gpsimd.dma_start(out_d[r, :], ob[:])

    for g in range(ngroups):
        t0, t1 = inflight[g % PREFETCH]
        if g == ngroups - 1 and g >= 1:
            # issue the previous group's phase 2 BEFORE the last group's
            # transposes so P(last-1) doesn't sit between T(last) and
            # P(last) in the PE queue at the very end
            phase2(g - 1)

        # Fold the 32 neighbour slices to 1 on the Vector engine.  The first
        # (biggest) fold level reads fp32 and writes bf16 into a separate
        # small tile; the remaining levels are all-bf16 in-place adds, which
        # run in the DVE 2x mode.  DVE work per tensor: 2048 fp32-rate
        # elems + 1920 bf16-rate elems (~3.5 us) vs 3968 fp32 (~4.9 us).
        # Each tile arrives as two half-transfers; the first 16 slices fold
        # while the second half is still in flight.  This hides the ~4 us
        # DMA completion latency every group and, crucially, cuts the
        # end-of-kernel tail: after the LAST byte lands only the B-half
        # fold (~1.8 us) remains instead of the full 7.8 us chain.
        pacc = ppool.tile([128, 3 * 128], F32, tag="pacc")
        sacc = spool.tile([128, 3 * 128], BF16, tag="sacc")
        h = 8 * F
        tbs = []
        for i, t in enumerate((t0, t1)):
            tb = tbpool.tile([128, 16 * F], BF16, tag=f"tb{i}")
            tbs.append(tb)
            nc.vector.tensor_add(tb[:, 0:h], t[:, 0:h], t[:, h:2 * h])
            for lv in (4, 2, 1):
                nc.vector.tensor_add(tb[:, 0:lv * F], tb[:, 0:lv * F],
                                     tb[:, lv * F:2 * lv * F])
        for i, t in enumerate((t0, t1)):
            tb = tbs[i]
            nc.vector.tensor_add(tb[:, h:2 * h], t[:, 2 * h:3 * h],
                                 t[:, 3 * h:4 * h])
            for lv in (4, 2, 1):
                nc.vector.tensor_add(tb[:, h:h + lv * F],
                                     tb[:, h:h + lv * F],
                                     tb[:, h + lv * F:h + 2 * lv * F])
            nc.vector.tensor_add(tb[:, 0:F], tb[:, 0:F], tb[:, h:h + F])
            nc.tensor.matmul(pacc[:, i * 128:(i + 1) * 128],
                             tb[:, 0:F], identb[:],
                             start=True, stop=True)
        nc.tensor.matmul(pacc[:, 256:384], xsb[:, g, :], identb[:],
                         start=True, stop=True)
        # ONE copy per group AFTER all three transposes.  (With per-region
        # copies interleaved between the transposes, tile-granular dep
        # tracking created write-after-read false deps pacc->copy->pacc that
        # serialized the whole back-end at ~6.6us/group.)
        nc.scalar.copy(sacc[:], pacc[:])
        state[g] = sacc

        if g >= 1 and g != ngroups - 1:
            phase2(g - 1)
        if g + PREFETCH < ngroups:
            inflight[(g + PREFETCH) % PREFETCH] = issue_loads(g + PREFETCH)

    phase2(ngroups - 1)


def build_nc(ngroups=BH // GROUP):
    bh = ngroups * GROUP
    nc = bacc.Bacc("TRN2", target_bir_lowering=False, debug=False)
    xn0 = nc.dram_tensor("xn0", [bh, N * F], F32, kind="ExternalInput")
    xn1 = nc.dram_tensor("xn1", [bh, N * F], F32, kind="ExternalInput")
    xs = nc.dram_tensor("xs", [bh, F], F32, kind="ExternalInput")
    cpack = nc.dram_tensor("cpack", [128, 4 * 128], F32, kind="ExternalInput")
    bvec = nc.dram_tensor("bvec", [1, D], F32, kind="ExternalInput")
    out = nc.dram_tensor("out", [bh, D], F32, kind="ExternalOutput")

    ins = [t.ap() for t in (xn0, xn1, xs, cpack, bvec)]
    with tile.TileContext(nc) as tc:
        _tile_kernel(tc, [out.ap()], ins, ngroups)
    nc.compile()
    return nc


def make_in_maps(x_self, x_neigh_0, x_neigh_1, w_self, w_neigh_0, w_neigh_1, b):
    """Shard full inputs into per-core input maps (batch axis, 8 ways)."""
    x_self = np.ascontiguousarray(np.asarray(x_self, dtype=np.float32))
    x_neigh_0 = np.ascontiguousarray(np.asarray(x_neigh_0, dtype=np.float32))
    x_neigh_1 = np.ascontiguousarray(np.asarray(x_neigh_1, dtype=np.float32))
    scale = np.float32(1.0 / (N * NR))
    w_s = np.asarray(w_self, dtype=np.float32)
    w0 = np.asarray(w_neigh_0, dtype=np.float32) * scale
    w1 = np.asarray(w_neigh_1, dtype=np.float32) * scale
    bvec = np.ascontiguousarray(np.asarray(b, dtype=np.float32).reshape(1, D))
    ident = np.eye(128, dtype=np.float32)
    # identity + the three (pre-scaled) weight matrices packed into one
    # [128, 512] block: 2 KiB per partition in ONE transfer instead of four
    # 512 B-descriptor transfers
    cpack = np.ascontiguousarray(
        np.concatenate([ident, w_s, w0, w1], axis=1), dtype=np.float32)

    in_maps = []
    for c in range(NCORES):
        bs = slice(c * BSH, (c + 1) * BSH)
        in_maps.append({
            "xn0": np.ascontiguousarray(x_neigh_0[bs].reshape(BH, N * F)),
            "xn1": np.ascontiguousarray(x_neigh_1[bs].reshape(BH, N * F)),
            "xs": np.ascontiguousarray(x_self[bs].reshape(BH, F)),
            "cpack": cpack, "bvec": bvec,
        })
    return in_maps


_NC_CACHE = None


def kernel(x_self, x_neigh_0, x_neigh_1, w_self, w_neigh_0, w_neigh_1, b):
    global _NC_CACHE
    if _NC_CACHE is None:
        _NC_CACHE = build_nc()
    in_maps = make_in_maps(x_self, x_neigh_0, x_neigh_1,
                           w_self, w_neigh_0, w_neigh_1, b)
    res = bass_utils.run_bass_kernel_spmd(
        _NC_CACHE, in_maps, core_ids=list(range(NCORES)))
    out = np.concatenate([r["out"] for r in res.results], axis=0)
    return out.reshape(B, H, D)


# revision 34
# speedup vs baseline: 1.0305x; 1.0305x over previous
"""Trainium2 Bass kernel: MeanHinAggregator (GNN message passing).

Reference computation (per batch-head element bh):
    z_r  = mean_n(x_neigh_r[bh, n, :]) @ w_neigh_r          (r = 0, 1)
    out  = relu(concat(x_self[bh] @ w_self, (z0 + z1) / 2) + b)

Strategy (pure data parallel over 8 NeuronCores, batch axis sharded):
  * Per core: B_shard=128, H=10 -> 1280 rows, processed in 10 groups of 128.
  * Neighbour tiles are DMA'd in natural layout [128 bh-part, (n f) free]
    (16 KiB contiguous per partition -> full DMA bandwidth).  Loads are
    prefetched PREFETCH groups deep so the SDMA queues never drain; this is
    a memory-bound kernel (43.9 MB/core vs ~358 GB/s HBM/core -> ~123 us
    floor) so the whole game is keeping DMA streaming at full rate.
  * The mean-over-neighbours reduction: five in-place strided adds on the
    Vector engine fold the 32 neighbour slices to 1 (fp32 matmuls pay a
    double-pass on the PE, so DVE folding is cheaper per element), then one
    transposing matmul per operand (lhsT = data, rhs = identity) lands the
    sums in PSUM in the [f, bh] layout the projection matmuls need as lhsT.
  * Engine discipline to keep every queue free of head-of-line blocking:
      - DVE runs ONLY the folds.
      - The PSUM->SBUF copies run on the Scalar (ACT) engine.
      - Phase 2 of group g (projections + bias + ReLU + store) is issued one
        iteration later, after phase 1 of group g+1, so the PE queue never
        waits on a copy that itself waits on the PE.
      - No gpsimd: SWDGE would fight DVE for the shared SBUF port pair.
  * Projection: out[bh, d] = lhsT(sumT).T @ w.  The 1/(N*NR) mean scaling is
    folded into host-prescaled copies of w_neigh_*.  Bias is added with a
    K=1 matmul (lhsT = ones row, rhs = bias row) accumulating into PSUM.
  * Final ReLU on the Scalar engine (PSUM -> SBUF), then DMA out on the
    scalar HWDGE ring (sync ring carries t0/ts, scalar ring t1/stores).
"""

import numpy as np

import concourse.bacc as bacc
import concourse.bass as bass
import concourse.tile as tile
from concourse import bass_utils, mybir
from concourse._compat import with_exitstack

B, H, N, F = 1024, 10, 32, 128
HALF = 128
D = 2 * HALF
NR = 2
NCORES = 8
BSH = B // NCORES        # 128 batch rows per core
BH = BSH * H             # 1280 (bh rows per core)
GROUP = 128              # bh rows per group
F32 = mybir.dt.float32
BF16 = mybir.dt.bfloat16
PREFETCH = 5             # groups of loads kept in flight


@with_exitstack
def _tile_kernel(ctx, tc, outs, ins, ngroups):
    nc = tc.nc
    xn0, xn1, xs, cpack, bvec = ins
    (out_d,) = outs

    const = ctx.enter_context(tc.tile_pool(name="const", bufs=1))
    xpool = ctx.enter_context(tc.tile_pool(name="xp", bufs=PREFETCH))
    tbpool = ctx.enter_context(tc.tile_pool(name="tb", bufs=3))
    spool = ctx.enter_context(tc.tile_pool(name="sp", bufs=3))
    opool = ctx.enter_context(tc.tile_pool(name="op", bufs=3))
    ppool = ctx.enter_context(tc.tile_pool(name="ps", bufs=3, space="PSUM"))
    pout = ctx.enter_context(tc.tile_pool(name="po", bufs=3, space="PSUM"))

    def issue_loads(g):
        """Queue group g's neighbour loads on the two HWDGE rings (sync +
        scalar), alternating the stream<->ring pairing per group so the ACT
        sequencer's slower boot doesn't systematically delay one stream.
        Group 0 is split into half-tile transfers: its fold can then start
        on the first half while the second is still in flight (the ~4 us
        DMA completion latency is exposed only at startup)."""
        r = slice(g * GROUP, (g + 1) * GROUP)
        ea, eb = (nc.sync, nc.scalar) if g % 2 == 0 else (nc.scalar, nc.sync)
        t0 = xpool.tile([128, N * F], F32, tag="t0")
        t1 = xpool.tile([128, N * F], F32, tag="t1")
        if g == 0:
            # only group 0 is split into half-transfers (its fold is on the
            # startup critical path).  Splitting EVERY group was measured
            # slower: 2x the DMA count pressures the 8 completion-sem lanes
            # and throttles the issue pipeline, and the two streams are
            # staggered anyway so the last tile's fold is only ~3.9 us.
            hf = 16 * F
            ea.dma_start(t0[:, 0:hf], xn0[r, 0:hf])
            eb.dma_start(t1[:, 0:hf], xn1[r, 0:hf])
            ea.dma_start(t0[:, hf:2 * hf], xn0[r, hf:2 * hf])
            eb.dma_start(t1[:, hf:2 * hf], xn1[r, hf:2 * hf])
        else:
            ea.dma_start(t0[:], xn0[r, :])
            eb.dma_start(t1[:], xn1[r, :])
        return t0, t1

    # The very first thing on each ring is a fat 2 MiB load: the kernel's
    # critical path starts with group 0's data.  Everything small (x_self,
    # weights, identity, bias) is queued behind it and still lands well
    # before its first use.  x_self is loaded ONCE for all groups: its rows
    # are only 512 B per partition, and per-group 64 KiB transfers of 512 B
    # descriptors are descriptor-dominated (measured 9 us for 64 KiB when
    # contending with fat loads) — one batched transfer pays that cost once.
    inflight = [issue_loads(0)]

    # Everything on the PE path is bf16: fp32 matmuls cost a double
    # LDWEIGHTS+MATMUL pass, bf16 is single-pass.  The fp32 consts are
    # cast to bf16 once on the Scalar engine at startup (rel tolerance is
    # 2e-2; bf16 rounding contributes ~0.5%).
    #
    # All small transfers (packed consts, bias, x_self) go on the gpsimd
    # SWDGE queue: a third DMA queue that drains concurrently with the two
    # fat HWDGE rings.  On the rings, their small descriptors cost ~10 us
    # of ring time each while contending with 16 KiB-descriptor streams
    # (measured).  DVE is idle at startup, so SWDGE descriptor emission is
    # not port-blocked.
    cp = const.tile([128, 4 * 128], F32, tag="cpack")
    nc.gpsimd.dma_start(cp[:], cpack[:])
    ident = cp[:, 0:128]
    wS_t = cp[:, 128:256]
    w0_t = cp[:, 256:384]
    w1_t = cp[:, 384:512]
    b_t = const.tile([1, D], F32, tag="b")
    nc.gpsimd.dma_start(b_t[:], bvec[:])
    xsall = const.tile([128, BH // GROUP, F], F32, tag="xsall")
    nc.gpsimd.dma_start(xsall[:], xs.rearrange("(g p) f -> p g f", p=GROUP))

    inflight += [issue_loads(g) for g in range(1, min(PREFETCH, ngroups))]

    # bf16 casts AFTER the prefetch load issues: the ACT engine is also the
    # HWDGE sequencer for one ring, and these casts wait ~20 us for the
    # SWDGE consts to land — issued earlier they head-of-line block that
    # ring's load issues (measured 12 us of ring delay).
    identb = const.tile([128, 128], BF16, tag="identb")
    nc.scalar.copy(identb[:], ident)
    wSb = const.tile([128, HALF], BF16, tag="wSb")
    nc.scalar.copy(wSb[:], wS_t)
    w0b = const.tile([128, HALF], BF16, tag="w0b")
    nc.scalar.copy(w0b[:], w0_t)
    w1b = const.tile([128, HALF], BF16, tag="w1b")
    nc.scalar.copy(w1b[:], w1_t)
    bb = const.tile([1, D], BF16, tag="bb")
    nc.scalar.copy(bb[:], b_t[:])
    xsb = const.tile([128, BH // GROUP, F], BF16, tag="xsb")
    nc.scalar.copy(xsb[:], xsall[:])
    ones_t = const.tile([1, 128], BF16, tag="ones")
    nc.vector.memset(ones_t[:], 1.0)
    state = {}

    def phase2(h):
        """Projections + bias + ReLU + store for group h (issued one
        iteration late so the PE queue never stalls on the ACT copy)."""
        sacc = state.pop(h)
        r = slice(h * GROUP, (h + 1) * GROUP)
        po = pout.tile([128, D], F32, tag="po")
        nc.tensor.matmul(po[:, 0:HALF], sacc[:, 256:384], wSb[:],
                         start=True, stop=False)
        nc.tensor.matmul(po[:, 0:HALF], ones_t[:], bb[:, 0:HALF],
                         start=False, stop=True)
        nc.tensor.matmul(po[:, HALF:D], sacc[:, 0:128], w0b[:],
                         start=True, stop=False)
        nc.tensor.matmul(po[:, HALF:D], sacc[:, 128:256], w1b[:],
                         start=False, stop=False)
        nc.tensor.matmul(po[:, HALF:D], ones_t[:], bb[:, HALF:D],
                         start=False, stop=True)
        ob = opool.tile([128, D], F32, tag="ob")
        nc.scalar.activation(ob[:], po[:], mybir.ActivationFunctionType.Relu)
        # Mid-run stores go on the SWDGE queue: 1 KiB-descriptor stores cost
        # ~2 us of HWDGE ring time each and delay every later load.  The
        # last two stores happen after the load streams have drained, where
        # the HWDGE rings are empty and lower-latency than SWDGE emission.
        if h >= BH // GROUP - 2:
            eng = nc.sync if h % 2 == 0 else nc.scalar
            eng.dma_start(out_d[r, :], ob[:])
        else:
            nc.gpsimd.dma_start(out_d[r, :], ob[:])

    for g in range(ngroups):
        t0, t1 = inflight[g % PREFETCH]
        if g == ngroups - 1 and g >= 1:
            # issue the previous group's phase 2 BEFORE the last group's
            # transposes so P(last-1) doesn't sit between T(last) and
            # P(last) in the PE queue at the very end
            phase2(g - 1)

        # Fold the 32 neighbour slices to 1 on the Vector engine.  The first
        # (biggest) fold level reads fp32 and writes bf16 into a separate
        # small tile; the remaining levels are all-bf16 in-place adds, which
        # run in the DVE 2x mode.  DVE work per tensor: 2048 fp32-rate
        # elems + 1920 bf16-rate elems (~3.5 us) vs 3968 fp32 (~4.9 us).
        pacc = ppool.tile([128, 3 * 128], F32, tag="pacc")
        sacc = spool.tile([128, 3 * 128], BF16, tag="sacc")
        h = 8 * F
        tbs = []
        for i, t in enumerate((t0, t1)):
            tb = tbpool.tile([128, 16 * F], BF16, tag=f"tb{i}")
            tbs.append(tb)
            if g == 0:
                # group 0's tiles arrive as two half-transfers: fold the
                # first 16 slices while the second half is still in flight
                nc.vector.tensor_add(tb[:, 0:h], t[:, 0:h], t[:, h:2 * h])
                for lv in (4, 2, 1):
                    nc.vector.tensor_add(tb[:, 0:lv * F], tb[:, 0:lv * F],
                                         tb[:, lv * F:2 * lv * F])
            else:
                nc.vector.tensor_add(tb[:], t[:, 0:16 * F],
                                     t[:, 16 * F:32 * F])
                for lv in (8, 4, 2, 1):
                    nc.vector.tensor_add(tb[:, 0:lv * F], tb[:, 0:lv * F],
                                         tb[:, lv * F:2 * lv * F])
        for i, t in enumerate((t0, t1)):
            tb = tbs[i]
            if g == 0:
                nc.vector.tensor_add(tb[:, h:2 * h], t[:, 2 * h:3 * h],
                                     t[:, 3 * h:4 * h])
                for lv in (4, 2, 1):
                    nc.vector.tensor_add(tb[:, h:h + lv * F],
                                         tb[:, h:h + lv * F],
                                         tb[:, h + lv * F:h + 2 * lv * F])
                nc.vector.tensor_add(tb[:, 0:F], tb[:, 0:F], tb[:, h:h + F])
            nc.tensor.matmul(pacc[:, i * 128:(i + 1) * 128],
                             tb[:, 0:F], identb[:],
                             start=True, stop=True)
        nc.tensor.matmul(pacc[:, 256:384], xsb[:, g, :], identb[:],
                         start=True, stop=True)
        # ONE copy per group AFTER all three transposes.  (With per-region
        # copies interleaved between the transposes, tile-granular dep
        # tracking created write-after-read false deps pacc->copy->pacc that
        # serialized the whole back-end at ~6.6us/group.)
        nc.scalar.copy(sacc[:], pacc[:])
        state[g] = sacc

        if g >= 1 and g != ngroups - 1:
            phase2(g - 1)
        if g + PREFETCH < ngroups:
            inflight[(g + PREFETCH) % PREFETCH] = issue_loads(g + PREFETCH)

    phase2(ngroups - 1)


def build_nc(ngroups=BH // GROUP):
    bh = ngroups * GROUP
    nc = bacc.Bacc("TRN2", target_bir_lowering=False, debug=False)
    xn0 = nc.dram_tensor("xn0", [bh, N * F], F32, kind="ExternalInput")
    xn1 = nc.dram_tensor("xn1", [bh, N * F], F32, kind="ExternalInput")
    xs = nc.dram_tensor("xs", [bh, F], F32, kind="ExternalInput")
    cpack = nc.dram_tensor("cpack", [128, 4 * 128], F32, kind="ExternalInput")
    bvec = nc.dram_tensor("bvec", [1, D], F32, kind="ExternalInput")
    out = nc.dram_tensor("out", [bh, D], F32, kind="ExternalOutput")

    ins = [t.ap() for t in (xn0, xn1, xs, cpack, bvec)]
    with tile.TileContext(nc) as tc:
        _tile_kernel(tc, [out.ap()], ins, ngroups)
    nc.compile()
    return nc


def make_in_maps(x_self, x_neigh_0, x_neigh_1, w_self, w_neigh_0, w_neigh_1, b):
    """Shard full inputs into per-core input maps (batch axis, 8 ways)."""
    x_self = np.ascontiguousarray(np.asarray(x_self, dtype=np.float32))
    x_neigh_0 = np.ascontiguousarray(np.asarray(x_neigh_0, dtype=np.float32))
    x_neigh_1 = np.ascontiguousarray(np.asarray(x_neigh_1, dtype=np.float32))
    scale = np.float32(1.0 / (N * NR))
    w_s = np.asarray(w_self, dtype=np.float32)
    w0 = np.asarray(w_neigh_0, dtype=np.float32) * scale
    w1 = np.asarray(w_neigh_1, dtype=np.float32) * scale
    bvec = np.ascontiguousarray(np.asarray(b, dtype=np.float32).reshape(1, D))
    ident = np.eye(128, dtype=np.float32)
    # identity + the three (pre-scaled) weight matrices packed into one
    # [128, 512] block: 2 KiB per partition in ONE transfer instead of four
    # 512 B-descriptor transfers
    cpack = np.ascontiguousarray(
        np.concatenate([ident, w_s, w0, w1], axis=1), dtype=np.float32)

    in_maps = []
    for c in range(NCORES):
        bs = slice(c * BSH, (c + 1) * BSH)
        in_maps.append({
            "xn0": np.ascontiguousarray(x_neigh_0[bs].reshape(BH, N * F)),
            "xn1": np.ascontiguousarray(x_neigh_1[bs].reshape(BH, N * F)),
            "xs": np.ascontiguousarray(x_self[bs].reshape(BH, F)),
            "cpack": cpack, "bvec": bvec,
        })
    return in_maps


_NC_CACHE = None


def kernel(x_self, x_neigh_0, x_neigh_1, w_self, w_neigh_0, w_neigh_1, b):
    global _NC_CACHE
    if _NC_CACHE is None:
        _NC_CACHE = build_nc()
    in_maps = make_in_maps(x_self, x_neigh_0, x_neigh_1,
                           w_self, w_neigh_0, w_neigh_1, b)
    res = bass_utils.run_bass_kernel_spmd(
        _NC_CACHE, in_maps, core_ids=list(range(NCORES)))
    out = np.concatenate([r["out"] for r in res.results], axis=0)
    return out.reshape(B, H, D)


# revision 35
# speedup vs baseline: 1.0479x; 1.0169x over previous
"""Trainium2 Bass kernel: MeanHinAggregator (GNN message passing).

Reference computation (per batch-head element bh):
    z_r  = mean_n(x_neigh_r[bh, n, :]) @ w_neigh_r          (r = 0, 1)
    out  = relu(concat(x_self[bh] @ w_self, (z0 + z1) / 2) + b)

Strategy (pure data parallel over 8 NeuronCores, batch axis sharded):
  * Per core: B_shard=128, H=10 -> 1280 rows, processed in 10 groups of 128.
  * Neighbour tiles are DMA'd in natural layout [128 bh-part, (n f) free]
    (16 KiB contiguous per partition -> full DMA bandwidth).  Loads are
    prefetched PREFETCH groups deep so the SDMA queues never drain; this is
    a memory-bound kernel (43.9 MB/core vs ~358 GB/s HBM/core -> ~123 us
    floor) so the whole game is keeping DMA streaming at full rate.
  * The mean-over-neighbours reduction: five in-place strided adds on the
    Vector engine fold the 32 neighbour slices to 1 (fp32 matmuls pay a
    double-pass on the PE, so DVE folding is cheaper per element), then one
    transposing matmul per operand (lhsT = data, rhs = identity) lands the
    sums in PSUM in the [f, bh] layout the projection matmuls need as lhsT.
  * Engine discipline to keep every queue free of head-of-line blocking:
      - DVE runs ONLY the folds.
      - The PSUM->SBUF copies run on the Scalar (ACT) engine.
      - Phase 2 of group g (projections + bias + ReLU + store) is issued one
        iteration later, after phase 1 of group g+1, so the PE queue never
        waits on a copy that itself waits on the PE.
      - No gpsimd: SWDGE would fight DVE for the shared SBUF port pair.
  * Projection: out[bh, d] = lhsT(sumT).T @ w.  The 1/(N*NR) mean scaling is
    folded into host-prescaled copies of w_neigh_*.  Bias is added with a
    K=1 matmul (lhsT = ones row, rhs = bias row) accumulating into PSUM.
  * Final ReLU on the Scalar engine (PSUM -> SBUF), then DMA out on the
    scalar HWDGE ring (sync ring carries t0/ts, scalar ring t1/stores).
"""

import numpy as np

import concourse.bacc as bacc
import concourse.bass as bass
import concourse.tile as tile
from concourse import bass_utils, mybir
from concourse._compat import with_exitstack

B, H, N, F = 1024, 10, 32, 128
HALF = 128
D = 2 * HALF
NR = 2
NCORES = 8
BSH = B // NCORES        # 128 batch rows per core
BH = BSH * H             # 1280 (bh rows per core)
GROUP = 128              # bh rows per group
F32 = mybir.dt.float32
BF16 = mybir.dt.bfloat16
PREFETCH = 5             # groups of loads kept in flight


@with_exitstack
def _tile_kernel(ctx, tc, outs, ins, ngroups):
    nc = tc.nc
    xn0, xn1, xs, cpack, bvec = ins
    (out_d,) = outs

    const = ctx.enter_context(tc.tile_pool(name="const", bufs=1))
    xpool = ctx.enter_context(tc.tile_pool(name="xp", bufs=PREFETCH))
    tbpool = ctx.enter_context(tc.tile_pool(name="tb", bufs=3))
    spool = ctx.enter_context(tc.tile_pool(name="sp", bufs=3))
    opool = ctx.enter_context(tc.tile_pool(name="op", bufs=3))
    ppool = ctx.enter_context(tc.tile_pool(name="ps", bufs=3, space="PSUM"))
    pout = ctx.enter_context(tc.tile_pool(name="po", bufs=3, space="PSUM"))

    def issue_loads(g):
        """Queue group g's neighbour loads on the two HWDGE rings (sync +
        scalar), alternating the stream<->ring pairing per group so the ACT
        sequencer's slower boot doesn't systematically delay one stream.
        Group 0 is split into half-tile transfers: its fold can then start
        on the first half while the second is still in flight (the ~4 us
        DMA completion latency is exposed only at startup)."""
        r = slice(g * GROUP, (g + 1) * GROUP)
        ea, eb = (nc.sync, nc.scalar) if g % 2 == 0 else (nc.scalar, nc.sync)
        t0 = xpool.tile([128, N * F], F32, tag="t0")
        t1 = xpool.tile([128, N * F], F32, tag="t1")
        if g in (0, ngroups - 1):
            # only the first and last groups are split into half-transfers
            # (group 0's fold is on the startup critical path; the last
            # group's B-half fold is the only fold work left after the
            # final byte lands, cutting the tail by ~2 us).  Splitting EVERY group was measured
            # slower: 2x the DMA count pressures the 8 completion-sem lanes
            # and throttles the issue pipeline, and the two streams are
            # staggered anyway so the last tile's fold is only ~3.9 us.
            hf = 16 * F
            ea.dma_start(t0[:, 0:hf], xn0[r, 0:hf])
            eb.dma_start(t1[:, 0:hf], xn1[r, 0:hf])
            ea.dma_start(t0[:, hf:2 * hf], xn0[r, hf:2 * hf])
            eb.dma_start(t1[:, hf:2 * hf], xn1[r, hf:2 * hf])
        else:
            ea.dma_start(t0[:], xn0[r, :])
            eb.dma_start(t1[:], xn1[r, :])
        return t0, t1

    # The very first thing on each ring is a fat 2 MiB load: the kernel's
    # critical path starts with group 0's data.  Everything small (x_self,
    # weights, identity, bias) is queued behind it and still lands well
    # before its first use.  x_self is loaded ONCE for all groups: its rows
    # are only 512 B per partition, and per-group 64 KiB transfers of 512 B
    # descriptors are descriptor-dominated (measured 9 us for 64 KiB when
    # contending with fat loads) — one batched transfer pays that cost once.
    inflight = [issue_loads(0)]

    # Everything on the PE path is bf16: fp32 matmuls cost a double
    # LDWEIGHTS+MATMUL pass, bf16 is single-pass.  The fp32 consts are
    # cast to bf16 once on the Scalar engine at startup (rel tolerance is
    # 2e-2; bf16 rounding contributes ~0.5%).
    #
    # All small transfers (packed consts, bias, x_self) go on the gpsimd
    # SWDGE queue: a third DMA queue that drains concurrently with the two
    # fat HWDGE rings.  On the rings, their small descriptors cost ~10 us
    # of ring time each while contending with 16 KiB-descriptor streams
    # (measured).  DVE is idle at startup, so SWDGE descriptor emission is
    # not port-blocked.
    cp = const.tile([128, 4 * 128], F32, tag="cpack")
    nc.gpsimd.dma_start(cp[:], cpack[:])
    ident = cp[:, 0:128]
    wS_t = cp[:, 128:256]
    w0_t = cp[:, 256:384]
    w1_t = cp[:, 384:512]
    b_t = const.tile([1, D], F32, tag="b")
    nc.gpsimd.dma_start(b_t[:], bvec[:])
    xsall = const.tile([128, BH // GROUP, F], F32, tag="xsall")
    nc.gpsimd.dma_start(xsall[:], xs.rearrange("(g p) f -> p g f", p=GROUP))

    inflight += [issue_loads(g) for g in range(1, min(PREFETCH, ngroups))]

    # bf16 casts AFTER the prefetch load issues: the ACT engine is also the
    # HWDGE sequencer for one ring, and these casts wait ~20 us for the
    # SWDGE consts to land — issued earlier they head-of-line block that
    # ring's load issues (measured 12 us of ring delay).
    identb = const.tile([128, 128], BF16, tag="identb")
    nc.scalar.copy(identb[:], ident)
    wSb = const.tile([128, HALF], BF16, tag="wSb")
    nc.scalar.copy(wSb[:], wS_t)
    w0b = const.tile([128, HALF], BF16, tag="w0b")
    nc.scalar.copy(w0b[:], w0_t)
    w1b = const.tile([128, HALF], BF16, tag="w1b")
    nc.scalar.copy(w1b[:], w1_t)
    bb = const.tile([1, D], BF16, tag="bb")
    nc.scalar.copy(bb[:], b_t[:])
    xsb = const.tile([128, BH // GROUP, F], BF16, tag="xsb")
    nc.scalar.copy(xsb[:], xsall[:])
    ones_t = const.tile([1, 128], BF16, tag="ones")
    nc.vector.memset(ones_t[:], 1.0)
    state = {}

    def phase2(h):
        """Projections + bias + ReLU + store for group h (issued one
        iteration late so the PE queue never stalls on the ACT copy)."""
        sacc = state.pop(h)
        r = slice(h * GROUP, (h + 1) * GROUP)
        po = pout.tile([128, D], F32, tag="po")
        nc.tensor.matmul(po[:, 0:HALF], sacc[:, 256:384], wSb[:],
                         start=True, stop=False)
        nc.tensor.matmul(po[:, 0:HALF], ones_t[:], bb[:, 0:HALF],
                         start=False, stop=True)
        nc.tensor.matmul(po[:, HALF:D], sacc[:, 0:128], w0b[:],
                         start=True, stop=False)
        nc.tensor.matmul(po[:, HALF:D], sacc[:, 128:256], w1b[:],
                         start=False, stop=False)
        nc.tensor.matmul(po[:, HALF:D], ones_t[:], bb[:, HALF:D],
                         start=False, stop=True)
        ob = opool.tile([128, D], F32, tag="ob")
        nc.scalar.activation(ob[:], po[:], mybir.ActivationFunctionType.Relu)
        # Mid-run stores go on the SWDGE queue: 1 KiB-descriptor stores cost
        # ~2 us of HWDGE ring time each and delay every later load.  The
        # last two stores happen after the load streams have drained, where
        # the HWDGE rings are empty and lower-latency than SWDGE emission.
        if h >= BH // GROUP - 2:
            eng = nc.sync if h % 2 == 0 else nc.scalar
            eng.dma_start(out_d[r, :], ob[:])
        else:
            nc.gpsimd.dma_start(out_d[r, :], ob[:])

    for g in range(ngroups):
        t0, t1 = inflight[g % PREFETCH]
        if g == ngroups - 1 and g >= 1:
            # issue the previous group's phase 2 BEFORE the last group's
            # transposes so P(last-1) doesn't sit between T(last) and
            # P(last) in the PE queue at the very end
            phase2(g - 1)

        # Fold the 32 neighbour slices to 1 on the Vector engine.  The first
        # (biggest) fold level reads fp32 and writes bf16 into a separate
        # small tile; the remaining levels are all-bf16 in-place adds, which
        # run in the DVE 2x mode.  DVE work per tensor: 2048 fp32-rate
        # elems + 1920 bf16-rate elems (~3.5 us) vs 3968 fp32 (~4.9 us).
        pacc = ppool.tile([128, 3 * 128], F32, tag="pacc")
        sacc = spool.tile([128, 3 * 128], BF16, tag="sacc")
        h = 8 * F
        tbs = []
        for i, t in enumerate((t0, t1)):
            tb = tbpool.tile([128, 16 * F], BF16, tag=f"tb{i}")
            tbs.append(tb)
            if g in (0, ngroups - 1):
                # these tiles arrive as two half-transfers: fold the
                # first 16 slices while the second half is still in flight
                nc.vector.tensor_add(tb[:, 0:h], t[:, 0:h], t[:, h:2 * h])
                for lv in (4, 2, 1):
                    nc.vector.tensor_add(tb[:, 0:lv * F], tb[:, 0:lv * F],
                                         tb[:, lv * F:2 * lv * F])
            else:
                nc.vector.tensor_add(tb[:], t[:, 0:16 * F],
                                     t[:, 16 * F:32 * F])
                for lv in (8, 4, 2, 1):
                    nc.vector.tensor_add(tb[:, 0:lv * F], tb[:, 0:lv * F],
                                         tb[:, lv * F:2 * lv * F])
        for i, t in enumerate((t0, t1)):
            tb = tbs[i]
            if g in (0, ngroups - 1):
                nc.vector.tensor_add(tb[:, h:2 * h], t[:, 2 * h:3 * h],
                                     t[:, 3 * h:4 * h])
                for lv in (4, 2, 1):
                    nc.vector.tensor_add(tb[:, h:h + lv * F],
                                         tb[:, h:h + lv * F],
                                         tb[:, h + lv * F:h + 2 * lv * F])
                nc.vector.tensor_add(tb[:, 0:F], tb[:, 0:F], tb[:, h:h + F])
            nc.tensor.matmul(pacc[:, i * 128:(i + 1) * 128],
                             tb[:, 0:F], identb[:],
                             start=True, stop=True)
        nc.tensor.matmul(pacc[:, 256:384], xsb[:, g, :], identb[:],
                         start=True, stop=True)
        # ONE copy per group AFTER all three transposes.  (With per-region
        # copies interleaved between the transposes, tile-granular dep
        # tracking created write-after-read false deps pacc->copy->pacc that
        # serialized the whole back-end at ~6.6us/group.)
        nc.scalar.copy(sacc[:], pacc[:])
        state[g] = sacc

        if g >= 1 and g != ngroups - 1:
            phase2(g - 1)
        if g + PREFETCH < ngroups:
            inflight[(g + PREFETCH) % PREFETCH] = issue_loads(g + PREFETCH)

    phase2(ngroups - 1)


def build_nc(ngroups=BH // GROUP):
    bh = ngroups * GROUP
    nc = bacc.Bacc("TRN2", target_bir_lowering=False, debug=False)
    xn0 = nc.dram_tensor("xn0", [bh, N * F], F32, kind="ExternalInput")
    xn1 = nc.dram_tensor("xn1", [bh, N * F], F32, kind="ExternalInput")
    xs = nc.dram_tensor("xs", [bh, F], F32, kind="ExternalInput")
    cpack = nc.dram_tensor("cpack", [128, 4 * 128], F32, kind="ExternalInput")
    bvec = nc.dram_tensor("bvec", [1, D], F32, kind="ExternalInput")
    out = nc.dram_tensor("out", [bh, D], F32, kind="ExternalOutput")

    ins = [t.ap() for t in (xn0, xn1, xs, cpack, bvec)]
    with tile.TileContext(nc) as tc:
        _tile_kernel(tc, [out.ap()], ins, ngroups)
    nc.compile()
    return nc


def make_in_maps(x_self, x_neigh_0, x_neigh_1, w_self, w_neigh_0, w_neigh_1, b):
    """Shard full inputs into per-core input maps (batch axis, 8 ways)."""
    x_self = np.ascontiguousarray(np.asarray(x_self, dtype=np.float32))
    x_neigh_0 = np.ascontiguousarray(np.asarray(x_neigh_0, dtype=np.float32))
    x_neigh_1 = np.ascontiguousarray(np.asarray(x_neigh_1, dtype=np.float32))
    scale = np.float32(1.0 / (N * NR))
    w_s = np.asarray(w_self, dtype=np.float32)
    w0 = np.asarray(w_neigh_0, dtype=np.float32) * scale
    w1 = np.asarray(w_neigh_1, dtype=np.float32) * scale
    bvec = np.ascontiguousarray(np.asarray(b, dtype=np.float32).reshape(1, D))
    ident = np.eye(128, dtype=np.float32)
    # identity + the three (pre-scaled) weight matrices packed into one
    # [128, 512] block: 2 KiB per partition in ONE transfer instead of four
    # 512 B-descriptor transfers
    cpack = np.ascontiguousarray(
        np.concatenate([ident, w_s, w0, w1], axis=1), dtype=np.float32)

    in_maps = []
    for c in range(NCORES):
        bs = slice(c * BSH, (c + 1) * BSH)
        in_maps.append({
            "xn0": np.ascontiguousarray(x_neigh_0[bs].reshape(BH, N * F)),
            "xn1": np.ascontiguousarray(x_neigh_1[bs].reshape(BH, N * F)),
            "xs": np.ascontiguousarray(x_self[bs].reshape(BH, F)),
            "cpack": cpack, "bvec": bvec,
        })
    return in_maps


_NC_CACHE = None


def kernel(x_self, x_neigh_0, x_neigh_1, w_self, w_neigh_0, w_neigh_1, b):
    global _NC_CACHE
    if _NC_CACHE is None:
        _NC_CACHE = build_nc()
    in_maps = make_in_maps(x_self, x_neigh_0, x_neigh_1,
                           w_self, w_neigh_0, w_neigh_1, b)
    res = bass_utils.run_bass_kernel_spmd(
        _NC_CACHE, in_maps, core_ids=list(range(NCORES)))
    out = np.concatenate([r["out"] for r in res.results], axis=0)
    return out.reshape(B, H, D)
